# revision 1
# baseline (speedup 1.0000x reference)
"""HRR binding self-attention kernel for 8 trn2 NeuronCores.

Math: out = irfft(c * rfft(x) * cumsum_s(rfft(x))) @ w_out.T, c = queries*keyvalues.
rfft is linear, so the causal cumsum commutes into the frequency domain; irfft
is linear, so it fuses into the output Linear: out = qv^T @ GW with
GW = (c * Gf) @ w_out.T precomputed on host (the real filter c commutes with
the complex products and rides along for free).

The forward DFT uses a three-level decimation-in-frequency split with every
twiddle folded into host-precomputed matrices (twiddles depend only on the
contraction index, so they fold; the even/odd output interleave is absorbed
into GW's row order):
  EEE = rfft_256(uuu)          covers freqs 8m        (uuu = uu1+uu2)
  EEO = DFT'_256(uud)          covers freqs 8m+4      (uud = uu1-uu2)
  EO  = DFT'_512(ud)           covers freqs 4m+2      (ud = u1-u2, u = x1+x2)
  O   = DFT'_1024(x1-x2)       covers odd freqs
This cuts the DFT matmul count 256 -> 88 per slab; deeper splits hit
complex-input subbranches or output-set overlap (cross terms grow the output
contraction by exactly the DFT saving), so this is near the fixed point.

Sharding: 8 shards = (batch b in 0..3) x (seq half h in 0..1), 2048 tokens
each.  h=1 shards get the first half's contribution as an initial carry,
computed on host as rfft(x[b, :2048].sum(0)) (negligible).

New-basis packed spectrum (2048 rows = 16 chunks of 128): [EEE 256 | EEO 256 |
EO 512 | O 1024], each block packed Re-then-Im so complex multiplies pair
chunks on equal partitions: pairs (0,1) | (2,3) | (4,6),(5,7) |
(8,12)..(11,15), with a 2-row fixup for the DC/Nyquist slots (chunk 0 row 0,
chunk 1 row 0).  O-side pairs are emitted first: they depend only on the
first u/s prep op, so the DFT pipeline starts while the deeper EEE/EEO prep
chain is still running.

Per-core single pass over 4 slabs of 512 tokens (matmuls bf16, f32 PSUM):
  - u/s prep: 4 wide DVE adds/subs on the x-chunks;
  - transposed DFT: CS chunks stationary, u/s moving -> freq-major spectrum
    [pk, tok] straight into PSUM (no token-major intermediate, no transpose);
  - Q copied to SBUF (ACT), then tensor_tensor_scan runs the causal cumsum
    in-place in PSUM (f32 state, per-partition carry chained across slabs);
  - complex multiply per chunk-pair on DVE -> qv bf16;
  - output matmul qv (stationary) @ GW (moving) -> out rows, bf16 staging
    (host casts back to f32).
Emission interleaves slab s's DFT with slab s-1's output matmul so the PE
never idles; the reps loop uses For_i(staggered_reset=True) so iterations
overlap without an all-engine barrier.
"""

import sys

sys.path.insert(0, "/opt/trn_rl_repo")

import numpy as np
import ml_dtypes

import concourse.bass as bass
import concourse.bacc as bacc
import concourse.mybir as mybir
from concourse.tile import TileContext
from concourse.bass_utils import run_bass_kernel_spmd

BF16 = mybir.dt.bfloat16
F32 = mybir.dt.float32
ADD = mybir.AluOpType.add
BYP = mybir.AluOpType.bypass

P = 128
D = 2048  # model dims
T = 2048  # tokens per shard
ND = D // P  # 16 d-chunks
NPF = 16  # packed-frequency chunks
TSB = 512  # tokens per slab
NSLAB = T // TSB  # 4
NB = 4  # batch
NS = 4096  # full seq

bf16 = ml_dtypes.bfloat16

_CACHE = {}


def _build_nc(reps: int = 1):
    nc = bacc.Bacc("TRN2", target_bir_lowering=False, debug=False, num_devices=8)
    xT = nc.dram_tensor("xT", [NSLAB, P, ND, TSB], BF16, kind="ExternalInput")
    CSEEE = nc.dram_tensor("CSEEE", [2, P, 2, P], BF16, kind="ExternalInput")
    CSEEO = nc.dram_tensor("CSEEO", [2, P, 2, P], BF16, kind="ExternalInput")
    CSEO = nc.dram_tensor("CSEO", [4, P, 4, P], BF16, kind="ExternalInput")
    CSO = nc.dram_tensor("CSO", [8, P, 8, P], BF16, kind="ExternalInput")
    GW = nc.dram_tensor("GW", [P, NPF, D], BF16, kind="ExternalInput")
    C0 = nc.dram_tensor("C0", [P, NPF], F32, kind="ExternalInput")
    out = nc.dram_tensor("out", [T, D], BF16, kind="ExternalOutput")

    with TileContext(nc) as tc:
        with tc.tile_pool(name="misc", bufs=1) as misc:
            c0_sb = misc.tile([P, NPF], F32)
            nc.sync.dma_start(c0_sb[:], C0[:])

            import contextlib

            loop_ctx = (
                tc.For_i(0, reps, 1, staggered_reset=True)
                if reps > 1
                else contextlib.nullcontext()
            )
            with loop_ctx:
                _body(nc, tc, c0_sb, CSEEE, CSEEO, CSEO, CSO, GW, xT, out)
    nc.finalize()
    return nc


# O-side pairs first: they depend only on the first u/s prep op, so the DFT
# pipeline starts while the deeper EEE/EEO prep chain is still running.
PAIRS = [(8, 12), (9, 13), (10, 14), (11, 15), (0, 1), (2, 3), (4, 6), (5, 7)]


def _body(nc, tc, c0_sb, CSEEE, CSEEO, CSEO, CSO, GW, xT, out):
    with (
        tc.tile_pool(name="wts", bufs=1) as wpool,
        tc.tile_pool(name="xt", bufs=2) as xpool,
        tc.tile_pool(name="ut", bufs=1) as utpool,
        tc.tile_pool(name="uut", bufs=1) as uutpool,
        tc.tile_pool(name="us", bufs=2) as uspool,
        tc.tile_pool(name="qsb", bufs=3) as qpool,
        tc.tile_pool(name="qv", bufs=2) as qvpool,
        tc.tile_pool(name="carry", bufs=2) as cpool,
        tc.tile_pool(name="tmp", bufs=1) as tpool,
        tc.tile_pool(name="osb", bufs=3) as opool,
        tc.tile_pool(name="psD", bufs=6, space="PSUM") as psD,
        tc.tile_pool(name="psC", bufs=2, space="PSUM") as psC,
    ):
        cseee_sb = wpool.tile([P, 2, 2, P], BF16)
        for pf in range(2):
            nc.sync.dma_start(cseee_sb[:, pf], CSEEE[pf])
        cseeo_sb = wpool.tile([P, 2, 2, P], BF16)
        for pf in range(2):
            nc.sync.dma_start(cseeo_sb[:, pf], CSEEO[pf])
        cseo_sb = wpool.tile([P, 4, 4, P], BF16)
        for pf in range(4):
            nc.sync.dma_start(cseo_sb[:, pf], CSEO[pf])
        cso_sb = wpool.tile([P, 8, 8, P], BF16)
        for pf in range(8):
            nc.sync.dma_start(cso_sb[:, pf], CSO[pf])
        gw_sb = wpool.tile([P, NPF, D], BF16)
        for pf in range(NPF):
            nc.sync.dma_start(gw_sb[:, pf, :], GW[:, pf, :])

        carry_prev = None
        qv_prev = None
        for s in range(NSLAB + 1):
            if s < NSLAB:
                xt = xpool.tile([P, ND, TSB], BF16, tag="xt")
                for q in range(4):
                    nc.sync.dma_start(xt[:, 4 * q : 4 * q + 4, :], xT[s, :, 4 * q : 4 * q + 4, :])
                # us chunks: 0..1 = uuu, 2..3 = uud, 4..7 = ud, 8..15 = x1-x2
                # (u = x1+x2, uu = u1+u2, ud = u1-u2, uuu = uu1+uu2, ...)
                us = uspool.tile([P, NPF, TSB], BF16, tag="us")
                nc.vector.tensor_sub(us[:, 8:16, :], xt[:, 0:8, :], xt[:, 8:16, :])
                u_t = utpool.tile([P, 8, TSB], BF16, tag="ut")
                nc.vector.tensor_add(u_t[:], xt[:, 0:8, :], xt[:, 8:16, :])
                uu_t = uutpool.tile([P, 4, TSB], BF16, tag="uut")
                nc.vector.tensor_add(uu_t[:], u_t[:, 0:4, :], u_t[:, 4:8, :])
                nc.vector.tensor_sub(us[:, 4:8, :], u_t[:, 0:4, :], u_t[:, 4:8, :])
                nc.vector.tensor_add(us[:, 0:2, :], uu_t[:, 0:2, :], uu_t[:, 2:4, :])
                nc.vector.tensor_sub(us[:, 2:4, :], uu_t[:, 0:2, :], uu_t[:, 2:4, :])
                qv = qvpool.tile([P, NPF, TSB], BF16, tag="qv")
                carry_sb = cpool.tile([P, NPF], F32, tag="carry")
                Qp0 = None
                for ci, (pfa, pfb) in enumerate(PAIRS):
                    Qp = qpool.tile([P, 2, TSB], BF16, tag="Q")
                    if (pfa, pfb) == (0, 1):
                        Qp0 = Qp
                    psts = {}
                    for h, pf in enumerate((pfa, pfb)):
                        if pf < 2:  # EEE: contracts uuu
                            cs_ap, ob, ndc = cseee_sb[:, pf], 0, 2
                        elif pf < 4:  # EEO: contracts uud
                            cs_ap, ob, ndc = cseeo_sb[:, pf - 2], 2, 2
                        elif pf < 8:  # EO: contracts ud
                            cs_ap, ob, ndc = cseo_sb[:, pf - 4], 4, 4
                        else:  # O: contracts x1-x2
                            cs_ap, ob, ndc = cso_sb[:, pf - 8], 8, 8
                        pst = psD.tile([P, TSB], F32, tag="psD")
                        for dc in range(ndc):
                            nc.tensor.matmul(
                                pst[:],
                                cs_ap[:, dc, :],
                                us[:, ob + dc, :],
                                start=(dc == 0),
                                stop=(dc == ndc - 1),
                            )
                        nc.scalar.copy(Qp[:, h, :], pst[:])
                        init = (
                            c0_sb[:, pf : pf + 1]
                            if s == 0
                            else carry_prev[:, pf : pf + 1]
                        )
                        # op1=bypass: state = data0 + state; data1 ignored
                        nc.vector.tensor_tensor_scan(
                            pst[:], pst[:], Qp[:, h, :], init, ADD, BYP
                        )
                        nc.scalar.copy(carry_sb[:, pf : pf + 1], pst[:, TSB - 1 : TSB])
                        psts[h] = pst
                    SR, SI = psts[0], psts[1]
                    QR, QI = Qp[:, 0, :], Qp[:, 1, :]
                    t1 = tpool.tile([P, TSB], F32, tag="t1")
                    t2 = tpool.tile([P, TSB], F32, tag="t2")
                    nc.vector.tensor_mul(t1[:], QR, SR[:])
                    nc.vector.tensor_mul(t2[:], QI, SI[:])
                    nc.vector.tensor_sub(qv[:, pfa, :], t1[:], t2[:])
                    t3 = tpool.tile([P, TSB], F32, tag="t1")
                    t4 = tpool.tile([P, TSB], F32, tag="t2")
                    nc.vector.tensor_mul(t3[:], QR, SI[:])
                    nc.vector.tensor_mul(t4[:], QI, SR[:])
                    nc.vector.tensor_add(qv[:, pfb, :], t3[:], t4[:])
                    if (pfa, pfb) == (0, 1):
                        # DC (chunk 0 row 0) and Nyquist (chunk 1 row 0): purely real
                        nc.vector.tensor_mul(qv[0:1, 0, :], Qp0[0:1, 0, :], SR[0:1, :])
                        nc.vector.tensor_mul(qv[0:1, 1, :], Qp0[0:1, 1, :], SI[0:1, :])
                carry_prev = carry_sb

            if s > 0:
                for tb in range(TSB // P):
                    for e in range(4):
                        psc = psC.tile([P, 512], F32, tag="psC")
                        for pf in range(NPF):
                            nc.tensor.matmul(
                                psc[:],
                                qv_prev[:, pf, tb * P : (tb + 1) * P],
                                gw_sb[:, pf, e * 512 : (e + 1) * 512],
                                start=(pf == 0),
                                stop=(pf == NPF - 1),
                            )
                        osb = opool.tile([P, 512], BF16, tag="osb")
                        if e % 2 == 0:
                            nc.scalar.copy(osb[:], psc[:])
                        else:
                            nc.vector.tensor_copy(osb[:], psc[:])
                        r0 = (s - 1) * TSB + tb * P
                        nc.sync.dma_start(
                            out[r0 : r0 + P, e * 512 : (e + 1) * 512], osb[:]
                        )
            if s < NSLAB:
                qv_prev = qv


def _chunked(m):
    """[rows, cols] -> [P, rows//P, cols] with row r at [r % P, r // P]."""
    r, c = m.shape
    return np.ascontiguousarray(m.reshape(r // P, P, c).transpose(1, 0, 2))


def _pack_spec(re, im):
    """re[1025], im[1025] -> packed [2048]: re[0..1024] then im[1..1023]."""
    return np.concatenate([re, im[1:1024]])


def _constants():
    if "consts" in _CACHE:
        return _CACHE["consts"]
    H = D // 2
    d = np.arange(D, dtype=np.float64)
    f = np.arange(D // 2 + 1, dtype=np.float64)
    ang = 2.0 * np.pi / D * np.outer(d, f)  # [D, 1025]
    cos, sin = np.cos(ang), np.sin(ang)
    alpha = np.full(1025, 2.0)
    alpha[0] = alpha[1024] = 1.0
    Gf = np.concatenate(
        [(alpha[:, None] * cos.T) / D, (-2.0 * sin[:, 1:1024].T) / D], axis=0
    )  # [2048 std-packed, D]
    # Two-level DIF split: EE = rfft_512(uu) covers freqs 4m; EO =
    # twiddle-folded DFT_512(ud) covers 4m+2; O = twiddle-folded DFT_1024 of
    # (x1-x2) covers odd freqs. All twiddles live in the host matrices.
    Qd = D // 4
    E8 = D // 8
    d3 = np.arange(E8, dtype=np.float64)
    mEEE = np.arange(E8 // 2 + 1, dtype=np.float64)
    angEEE = 2.0 * np.pi / E8 * np.outer(d3, mEEE)
    CS_EEE = np.concatenate([np.cos(angEEE), -np.sin(angEEE[:, 1:128])], axis=1)
    mEEO = np.arange(128, dtype=np.float64)
    thEEO = np.pi / E8 * np.outer(d3, 2 * mEEO + 1)
    CS_EEO = np.concatenate([np.cos(thEEO), -np.sin(thEEO)], axis=1)
    d2 = np.arange(Qd, dtype=np.float64)
    mEO = np.arange(256, dtype=np.float64)
    thEO = np.pi / Qd * np.outer(d2, 2 * mEO + 1)
    CS_EO = np.concatenate([np.cos(thEO), -np.sin(thEO)], axis=1)
    d1 = np.arange(H, dtype=np.float64)
    mO = np.arange(512, dtype=np.float64)
    thO = np.pi / H * np.outer(d1, 2 * mO + 1)
    CS_O = np.concatenate([np.cos(thO), -np.sin(thO)], axis=1)
    # chunked [pf, p, dc, j] = mat[128*dc + p, 128*pf + j]
    CSEEE2 = np.ascontiguousarray(CS_EEE.reshape(2, P, 2, P).transpose(2, 1, 0, 3))
    CSEEO2 = np.ascontiguousarray(CS_EEO.reshape(2, P, 2, P).transpose(2, 1, 0, 3))
    CSEO2 = np.ascontiguousarray(CS_EO.reshape(4, P, 4, P).transpose(2, 1, 0, 3))
    CSO2 = np.ascontiguousarray(CS_O.reshape(8, P, 8, P).transpose(2, 1, 0, 3))
    # new-basis row order: [EEE 256; EEO 256; EO 512; O 1024] -> std-packed rows
    perm = np.empty(2048, np.int64)
    perm[0:129] = 8 * np.arange(129)
    perm[129:256] = 1024 + 8 * np.arange(1, 128)
    perm[256:384] = 8 * np.arange(128) + 4
    perm[384:512] = 1024 + 8 * np.arange(128) + 4
    perm[512:768] = 4 * np.arange(256) + 2
    perm[768:1024] = 1024 + 4 * np.arange(256) + 2
    perm[1024:1536] = 2 * np.arange(512) + 1
    perm[1536:2048] = 1025 + 2 * np.arange(512)
    consts = {
        "CSEEE2": CSEEE2.astype(np.float32).astype(bf16),
        "CSEEO2": CSEEO2.astype(np.float32).astype(bf16),
        "CSEO2": CSEO2.astype(np.float32).astype(bf16),
        "CSO2": CSO2.astype(np.float32).astype(bf16),
        "Gf": Gf,
        "perm": perm,
    }
    _CACHE["consts"] = consts
    return consts


def kernel(x, queries, keyvalues, w_out):
    x = np.asarray(x, dtype=np.float32)
    queries = np.asarray(queries, dtype=np.float32)
    keyvalues = np.asarray(keyvalues, dtype=np.float32)
    w_out = np.asarray(w_out, dtype=np.float32)

    if "nc" not in _CACHE:
        _CACHE["nc"] = _build_nc()
    nc = _CACHE["nc"]
    consts = _constants()

    c = (queries * keyvalues).reshape(-1)  # [1025]
    c_packed = _pack_spec(c, c)  # [2048] std-packed
    GWf = (c_packed[:, None] * consts["Gf"]).astype(np.float32) @ w_out.T
    GWc = _chunked(GWf[consts["perm"]].astype(np.float32)).astype(bf16)

    in_maps = []
    shards = []
    for b in range(NB):
        for h in range(2):
            shards.append((b, h))
            xs = x[b, h * T : (h + 1) * T]  # [T, D]
            xT3 = _chunked(np.ascontiguousarray(xs.T))  # [P, ND, T]
            xTc = np.ascontiguousarray(
                xT3.reshape(P, ND, NSLAB, TSB).transpose(2, 0, 1, 3)
            ).astype(bf16)
            if h == 0:
                c0 = np.zeros((P, NPF), np.float32)
            else:
                F = np.fft.rfft(x[b, :T].sum(axis=0).astype(np.float64))
                c0s = _pack_spec(F.real, F.imag).astype(np.float32)
                c0 = _chunked(c0s[consts["perm"]][:, None])[:, :, 0]
            in_maps.append(
                {
                    "xT": xTc,
                    "CSEEE": consts["CSEEE2"],
                    "CSEEO": consts["CSEEO2"],
                    "CSEO": consts["CSEO2"],
                    "CSO": consts["CSO2"],
                    "GW": GWc,
                    "C0": np.ascontiguousarray(c0),
                }
            )

    global _LAST_IN_MAPS
    _LAST_IN_MAPS = in_maps
    res = run_bass_kernel_spmd(nc, in_maps, core_ids=list(range(8)))
    y = np.empty((NB, NS, D), np.float32)
    for i, (b, h) in enumerate(shards):
        y[b, h * T : (h + 1) * T] = res.results[i]["out"].astype(np.float32)
    return y



# revision 2
# speedup vs baseline: 1.0114x; 1.0114x over previous
"""HRR binding self-attention kernel for 8 trn2 NeuronCores.

Math: out = irfft(c * rfft(x) * cumsum_s(rfft(x))) @ w_out.T, c = queries*keyvalues.
rfft is linear, so the causal cumsum commutes into the frequency domain; irfft
is linear, so it fuses into the output Linear: out = qv^T @ GW with
GW = (c * Gf) @ w_out.T precomputed on host.

The forward DFT uses a three-level decimation-in-frequency split with every
twiddle folded into host-precomputed matrices:
  EEE = rfft_256(uuu)          covers freqs 8m        (uuu = uu1+uu2)
  EEO = DFT'_256(uud)          covers freqs 8m+4      (uud = uu1-uu2)
  EO  = DFT'_512(ud)           covers freqs 4m+2      (ud = u1-u2, u = x1+x2)
  O   = DFT'_1024(x1-x2)       covers odd freqs

Hybrid precision: everything fp16 (tensor engine runs fp16 at bf16 rate, and
fp16's m10 cuts the quantization error ~8x vs bf16), EXCEPT low-spectral-weight
basis rows which run as fp8e4 DoubleRow matmuls at 2x rate. The basis rows are
re-sorted within each DFT block by |c_f|^2 so the heavy frequencies concentrate
in designated fp16 chunk pairs; c^2 is so heavy-tailed that the fp8 pairs carry
only ~4.5% of the output norm (adds ~9e-3 rel err vs the 2e-2 budget).
DoubleRow packs two 128-row contraction chunks per instruction:
out += sum_i w[:,i].T @ m[:,i] with w [K,2,M], m [K,2,N] slot-major.

Per-frequency scales lambda^2 (folded host-side into CS columns, C0 carry and
GW rows) keep both fp8 operands inside e4m3's +-240 range; the Q copy applies
a power-of-2 beta per chunk (2^-6 fp16 / 2^-15 fp8) so qv fits fp16/fp8e4,
inverted via GW. Output staged fp16 scaled by 2^-2; host multiplies back.

Sharding: 8 shards = (batch b in 0..3) x (seq half h in 0..1), 2048 tokens
each. h=1 shards get the first half's contribution as an initial carry,
computed on host as rfft(x[b, :2048].sum(0)).

Chunk pairs (Re-chunk, Im-chunk) at equal partition positions:
  pair 0 (0,1)=EEE f=8m [c-sorted, DC/Nyq pinned at pos 0], 1 (2,3)=EEO 8m+4,
  2 (4,6)/3 (5,7)=EO 4m+2 split by c-rank, 4..7 (8,12)(9,13)(10,14)(11,15)=O
  odd freqs by c-rank. GEMM fp8 pairs: {1,3,5,6,7}; DFT fp8 pairs: {3,6,7}.

Per-core single pass over 4 slabs of 512 tokens; emission interleaves slab
s's DFT with slab s-1's output matmul so the PE never idles."""

import sys

sys.path.insert(0, "/opt/trn_rl_repo")

import hashlib

import numpy as np
import ml_dtypes

import concourse.bass as bass
import concourse.bacc as bacc
import concourse.mybir as mybir
from concourse.tile import TileContext
from concourse.bass_utils import run_bass_kernel_spmd

F16 = mybir.dt.float16
FP8 = mybir.dt.float8e4
F32 = mybir.dt.float32
ADD = mybir.AluOpType.add
BYP = mybir.AluOpType.bypass
DR = mybir.MatmulPerfMode.DoubleRow

P = 128
D = 2048
T = 2048
ND = D // P
NPF = 16
TSB = 512
NSLAB = T // TSB
NB = 4
NS = 4096
NF = D // 2 + 1

f16 = np.float16
e4 = ml_dtypes.float8_e4m3

# ---- hybrid precision config ----
PAIR_CHUNKS = [(0, 1), (2, 3), (4, 6), (5, 7), (8, 12), (9, 13), (10, 14), (11, 15)]
PAIR_BLOCK = ["EEE", "EEO", "EO", "EO", "O", "O", "O", "O"]
BLOCK_US = {"EEE": (0, 2), "EEO": (2, 2), "EO": (4, 4), "O": (8, 8)}
GEMM_F8 = (1, 3, 5, 6, 7)  # pair indices with fp8 output GEMM
DFT_F8 = (3, 6, 7)  # pair indices with fp8 DFT (subset of GEMM_F8)
PAIRS_16 = (0, 2, 4)  # fp16 GEMM pairs
CH16 = [0, 1, 4, 6, 8, 12]  # chunks of PAIRS_16, GW16/qv16 order
CH16_IDX = {ch: i for i, ch in enumerate(CH16)}
P8_IDX = {pi: j for j, pi in enumerate(GEMM_F8)}
N16 = len(CH16)
N8 = len(GEMM_F8)
B16S = 2.0 ** -6  # Q-copy scale, fp16 pairs
B8S = 2.0 ** -15  # Q-copy scale, fp8 pairs
OUT_SC = 2.0 ** -2  # folded into GW; host multiplies output back
GW_TGT = 200.0  # target max of fp8 GW rows (<=240)

# CS tensor chunk orders (tile index -> chunk id)
CS_O16_CH = [8, 12, 9, 13]
CS_EO8_CH = [5, 7]
CS_O8_CH = [10, 14, 11, 15]

_CACHE = {}


def _build_nc(reps: int = 1):
    nc = bacc.Bacc("TRN2", target_bir_lowering=False, debug=False, num_devices=8)
    xT = nc.dram_tensor("xT", [NSLAB, P, ND, TSB], F16, kind="ExternalInput")
    CSEEE = nc.dram_tensor("CSEEE", [2, P, 2, P], F16, kind="ExternalInput")
    CSEEO = nc.dram_tensor("CSEEO", [2, P, 2, P], F16, kind="ExternalInput")
    CSEO16 = nc.dram_tensor("CSEO16", [2, P, 4, P], F16, kind="ExternalInput")
    CSO16 = nc.dram_tensor("CSO16", [4, P, 8, P], F16, kind="ExternalInput")
    CSEO8 = nc.dram_tensor("CSEO8", [2, P, 2, 2, P], FP8, kind="ExternalInput")
    CSO8 = nc.dram_tensor("CSO8", [4, P, 4, 2, P], FP8, kind="ExternalInput")
    GW16 = nc.dram_tensor("GW16", [P, N16, D], F16, kind="ExternalInput")
    GW8 = nc.dram_tensor("GW8", [P, N8, 2, D], FP8, kind="ExternalInput")
    C0 = nc.dram_tensor("C0", [P, NPF], F32, kind="ExternalInput")
    out = nc.dram_tensor("out", [T, D], F16, kind="ExternalOutput")

    with TileContext(nc) as tc:
        with (
            tc.tile_pool(name="misc", bufs=1) as misc,
            tc.tile_pool(name="wts", bufs=1) as wpool,
        ):
            c0_sb = misc.tile([P, NPF], F32)
            nc.sync.dma_start(c0_sb[:], C0[:])

            # Loop-invariant weight loads, hoisted out of the reps loop.
            # CS first (needed by the first DFT), GW per e-block last (each
            # 512-col GEMM group only waits on its own quarter).
            cseee_sb = wpool.tile([P, 2, 2, P], F16)
            for pf in range(2):
                nc.sync.dma_start(cseee_sb[:, pf], CSEEE[pf])
            cseeo_sb = wpool.tile([P, 2, 2, P], F16)
            for pf in range(2):
                nc.sync.dma_start(cseeo_sb[:, pf], CSEEO[pf])
            cseo16_sb = wpool.tile([P, 2, 4, P], F16)
            for pf in range(2):
                nc.sync.dma_start(cseo16_sb[:, pf], CSEO16[pf])
            cso16_sb = wpool.tile([P, 4, 8, P], F16)
            for pf in range(4):
                nc.sync.dma_start(cso16_sb[:, pf], CSO16[pf])
            cseo8_sb = wpool.tile([P, 2, 2, 2, P], FP8)
            for pf in range(2):
                nc.sync.dma_start(cseo8_sb[:, pf], CSEO8[pf])
            cso8_sb = wpool.tile([P, 4, 4, 2, P], FP8)
            for pf in range(4):
                nc.sync.dma_start(cso8_sb[:, pf], CSO8[pf])
            gw16_e = []
            gw8_e = []
            for e in range(4):
                g16 = wpool.tile([P, N16, 512], F16, tag=f"gw16_{e}")
                for i in range(N16):
                    nc.sync.dma_start(g16[:, i, :], GW16[:, i, e * 512 : (e + 1) * 512])
                gw16_e.append(g16)
                g8 = wpool.tile([P, N8, 2, 512], FP8, tag=f"gw8_{e}")
                for j in range(N8):
                    nc.sync.dma_start(g8[:, j], GW8[:, j, :, e * 512 : (e + 1) * 512])
                gw8_e.append(g8)

            # cs lookup: chunk id -> (tile, index)
            cs16 = {}
            cs16[0], cs16[1] = (cseee_sb, 0), (cseee_sb, 1)
            cs16[2], cs16[3] = (cseeo_sb, 0), (cseeo_sb, 1)
            cs16[4], cs16[6] = (cseo16_sb, 0), (cseo16_sb, 1)
            for i, ch in enumerate(CS_O16_CH):
                cs16[ch] = (cso16_sb, i)
            cs8 = {}
            for i, ch in enumerate(CS_EO8_CH):
                cs8[ch] = (cseo8_sb, i)
            for i, ch in enumerate(CS_O8_CH):
                cs8[ch] = (cso8_sb, i)

            import contextlib

            loop_ctx = (
                tc.For_i(0, reps, 1, staggered_reset=True)
                if reps > 1
                else contextlib.nullcontext()
            )
            with loop_ctx:
                _body(nc, tc, c0_sb, cs16, cs8, gw16_e, gw8_e, xT, out)
    nc.finalize()
    return nc


# O-side pairs first: they depend only on the first u/s prep op, so the DFT
# pipeline starts while the deeper EEE/EEO prep chain is still running.
PAIR_ORDER = [4, 5, 6, 7, 0, 1, 2, 3]


def _body(nc, tc, c0_sb, cs16, cs8, gw16_e, gw8_e, xT, out):
    with (
        tc.tile_pool(name="xt", bufs=2) as xpool,
        tc.tile_pool(name="ut", bufs=1) as utpool,
        tc.tile_pool(name="uut", bufs=1) as uutpool,
        tc.tile_pool(name="us", bufs=2) as uspool,
        tc.tile_pool(name="us8", bufs=2) as us8pool,
        tc.tile_pool(name="qsb", bufs=3) as qpool,
        tc.tile_pool(name="qv", bufs=2) as qvpool,
        tc.tile_pool(name="qv8", bufs=2) as qv8pool,
        tc.tile_pool(name="carry", bufs=2) as cpool,
        tc.tile_pool(name="tmp", bufs=1) as tpool,
        tc.tile_pool(name="osb", bufs=3) as opool,
        tc.tile_pool(name="psD", bufs=6, space="PSUM") as psD,
        tc.tile_pool(name="psC", bufs=2, space="PSUM") as psC,
    ):
        carry_prev = None
        qv_prev = None
        qv8_prev = None
        for s in range(NSLAB + 1):
            if s < NSLAB:
                xt = xpool.tile([P, ND, TSB], F16, tag="xt")
                for qd in range(4):
                    nc.sync.dma_start(
                        xt[:, 4 * qd : 4 * qd + 4, :], xT[s, :, 4 * qd : 4 * qd + 4, :]
                    )
                # us chunks: 0..1 = uuu, 2..3 = uud, 4..7 = ud, 8..15 = x1-x2
                us = uspool.tile([P, NPF, TSB], F16, tag="us")
                nc.vector.tensor_sub(us[:, 8:16, :], xt[:, 0:8, :], xt[:, 8:16, :])
                u_t = utpool.tile([P, 8, TSB], F16, tag="ut")
                nc.vector.tensor_add(u_t[:], xt[:, 0:8, :], xt[:, 8:16, :])
                uu_t = uutpool.tile([P, 4, TSB], F16, tag="uut")
                nc.vector.tensor_add(uu_t[:], u_t[:, 0:4, :], u_t[:, 4:8, :])
                nc.vector.tensor_sub(us[:, 4:8, :], u_t[:, 0:4, :], u_t[:, 4:8, :])
                nc.vector.tensor_add(us[:, 0:2, :], uu_t[:, 0:2, :], uu_t[:, 2:4, :])
                nc.vector.tensor_sub(us[:, 2:4, :], uu_t[:, 0:2, :], uu_t[:, 2:4, :])
                # fp8 copies of us chunks 4..15 (EO + O contractions)
                us8 = us8pool.tile([P, 12, TSB], FP8, tag="us8")
                nc.vector.tensor_copy(us8[:, 4:12, :], us[:, 8:16, :])
                nc.vector.tensor_copy(us8[:, 0:4, :], us[:, 4:8, :])

                qv = qvpool.tile([P, N16, TSB], F16, tag="qv")
                qv8 = qv8pool.tile([P, N8, 2, TSB], FP8, tag="qv8")
                carry_sb = cpool.tile([P, NPF], F32, tag="carry")
                Qp0 = None
                for pi in PAIR_ORDER:
                    pfa, pfb = PAIR_CHUNKS[pi]
                    blk = PAIR_BLOCK[pi]
                    ob, ndc = BLOCK_US[blk]
                    isf8_dft = pi in DFT_F8
                    isf8_gemm = pi in GEMM_F8
                    beta = B8S if isf8_gemm else B16S
                    Qp = qpool.tile([P, 2, TSB], F16, tag="Q")
                    if pi == 0:
                        Qp0 = Qp
                    psts = {}
                    for h, pf in enumerate((pfa, pfb)):
                        pst = psD.tile([P, TSB], F32, tag="psD")
                        if isf8_dft:
                            tile8, idx8 = cs8[pf]
                            u8b = ob - 4  # us8 index base
                            for jdc in range(ndc // 2):
                                nc.tensor.matmul(
                                    pst[:],
                                    tile8[:, idx8, jdc],
                                    us8[:, u8b + 2 * jdc : u8b + 2 * jdc + 2, :],
                                    start=(jdc == 0),
                                    stop=(jdc == ndc // 2 - 1),
                                    perf_mode=DR,
                                )
                        else:
                            tile16, idx16 = cs16[pf]
                            for dc in range(ndc):
                                nc.tensor.matmul(
                                    pst[:],
                                    tile16[:, idx16, dc, :],
                                    us[:, ob + dc, :],
                                    start=(dc == 0),
                                    stop=(dc == ndc - 1),
                                )
                        nc.scalar.mul(Qp[:, h, :], pst[:], beta)
                        init = (
                            c0_sb[:, pf : pf + 1]
                            if s == 0
                            else carry_prev[:, pf : pf + 1]
                        )
                        # op1=bypass: state = data0 + state; data1 ignored
                        nc.vector.tensor_tensor_scan(
                            pst[:], pst[:], Qp[:, h, :], init, ADD, BYP
                        )
                        nc.scalar.copy(carry_sb[:, pf : pf + 1], pst[:, TSB - 1 : TSB])
                        psts[h] = pst
                    SR, SI = psts[0], psts[1]
                    QR, QI = Qp[:, 0, :], Qp[:, 1, :]
                    if isf8_gemm:
                        j8 = P8_IDX[pi]
                        dst_re = qv8[:, j8, 0, :]
                        dst_im = qv8[:, j8, 1, :]
                    else:
                        dst_re = qv[:, CH16_IDX[pfa], :]
                        dst_im = qv[:, CH16_IDX[pfb], :]
                    t1 = tpool.tile([P, TSB], F32, tag="t1")
                    t2 = tpool.tile([P, TSB], F32, tag="t2")
                    nc.vector.tensor_mul(t1[:], QR, SR[:])
                    nc.vector.tensor_mul(t2[:], QI, SI[:])
                    nc.vector.tensor_sub(dst_re, t1[:], t2[:])
                    t3 = tpool.tile([P, TSB], F32, tag="t1")
                    t4 = tpool.tile([P, TSB], F32, tag="t2")
                    nc.vector.tensor_mul(t3[:], QR, SI[:])
                    nc.vector.tensor_mul(t4[:], QI, SR[:])
                    nc.vector.tensor_add(dst_im, t3[:], t4[:])
                    if pi == 0:
                        # DC (chunk 0 pos 0) and Nyquist (chunk 1 pos 0): purely real
                        nc.vector.tensor_mul(qv[0:1, 0, :], Qp0[0:1, 0, :], SR[0:1, :])
                        nc.vector.tensor_mul(qv[0:1, 1, :], Qp0[0:1, 1, :], SI[0:1, :])
                carry_prev = carry_sb

            if s > 0:
                for tb in range(TSB // P):
                    for e in range(4):
                        psc = psC.tile([P, 512], F32, tag="psC")
                        # interleave fp16 / DoubleRow insts so the 256-row
                        # DR stationary loads hide behind fp16 streams
                        plan = []
                        for k in range(max(N16, N8)):
                            if k < N16:
                                plan.append((False, k))
                            if k < N8:
                                plan.append((True, k))
                        for n, (is8, k) in enumerate(plan):
                            if is8:
                                nc.tensor.matmul(
                                    psc[:],
                                    qv8_prev[:, k, :, tb * P : (tb + 1) * P],
                                    gw8_e[e][:, k],
                                    start=(n == 0),
                                    stop=(n == len(plan) - 1),
                                    perf_mode=DR,
                                )
                            else:
                                nc.tensor.matmul(
                                    psc[:],
                                    qv_prev[:, k, tb * P : (tb + 1) * P],
                                    gw16_e[e][:, k, :],
                                    start=(n == 0),
                                    stop=(n == len(plan) - 1),
                                )
                        osb = opool.tile([P, 512], F16, tag="osb")
                        if e % 2 == 0:
                            nc.scalar.copy(osb[:], psc[:])
                        else:
                            nc.vector.tensor_copy(osb[:], psc[:])
                        r0 = (s - 1) * TSB + tb * P
                        nc.sync.dma_start(
                            out[r0 : r0 + P, e * 512 : (e + 1) * 512], osb[:]
                        )
            if s < NSLAB:
                qv_prev = qv
                qv8_prev = qv8


def _cs_cols(block, freqs):
    """CS columns for given output freqs of a block: cos cols then -sin cols.
    Maps the block's prepped input (uuu/uud/ud/x1-x2) to Re/Im of rfft."""
    f = np.asarray(freqs, dtype=np.float64)
    if block == "EEE":
        d = np.arange(256, dtype=np.float64)
        ang = 2.0 * np.pi / 256 * np.outer(d, f / 8)
    elif block == "EEO":
        d = np.arange(256, dtype=np.float64)
        ang = np.pi / 256 * np.outer(d, 2 * (f - 4) / 8 + 1)
    elif block == "EO":
        d = np.arange(512, dtype=np.float64)
        ang = np.pi / 512 * np.outer(d, 2 * (f - 2) / 4 + 1)
    else:
        d = np.arange(1024, dtype=np.float64)
        ang = np.pi / 1024 * np.outer(d, 2 * (f - 1) / 2 + 1)
    return np.cos(ang), -np.sin(ang)


def _chunk_cs(mat, npf, ndc):
    """[d_in, n_cols] -> [npf, P, ndc, P]: entry [pf,p,dc,j] = mat[128*dc+p, 128*pf+j]."""
    return np.ascontiguousarray(
        mat.reshape(ndc, P, npf, P).transpose(2, 1, 0, 3)
    )


def _chunk_cs8(mat, npf, ndc):
    """[d_in, n_cols] -> [npf, P, ndc//2, 2, P] with dc pairs in the slot dim."""
    a = mat.reshape(ndc // 2, 2, P, npf, P)  # [jdc, slot, p, pf, j]
    return np.ascontiguousarray(a.transpose(3, 2, 0, 1, 4))


def _constants(queries, keyvalues, w_out):
    key = hashlib.sha1(
        queries.tobytes() + keyvalues.tobytes() + w_out.tobytes()
    ).hexdigest()
    if _CACHE.get("ckey") == key:
        return _CACHE["consts"]

    c = (queries.reshape(-1).astype(np.float64)) * (
        keyvalues.reshape(-1).astype(np.float64)
    )
    alpha = np.full(NF, 2.0)
    alpha[0] = alpha[NF - 1] = 1.0
    wgt = (c * alpha) ** 2

    def srt(fs):
        fs = np.asarray(fs)
        return fs[np.argsort(-wgt[fs], kind="stable")]

    f_eee = srt(8 * np.arange(1, 128))
    f_eeo = srt(8 * np.arange(128) + 4)
    f_eo = srt(4 * np.arange(256) + 2)
    f_o = srt(2 * np.arange(512) + 1)
    pair_freqs = {
        0: f_eee,
        1: f_eeo,
        2: f_eo[:128],
        3: f_eo[128:],
        4: f_o[:128],
        5: f_o[128:256],
        6: f_o[256:384],
        7: f_o[384:],
    }

    # G synthesis rows (irfft columns) [NF, D] for Re and Im parts
    dfull = np.arange(D, dtype=np.float64)
    ff = np.arange(NF, dtype=np.float64)
    ang = 2.0 * np.pi / D * np.outer(ff, dfull)  # [NF, D]
    Gre = (alpha[:, None] * np.cos(ang)) / D
    Gim = (-2.0 * np.sin(ang)) / D

    w64 = w_out.astype(np.float64)
    # GW natural rows per freq: re_row[f] = c_f * Gre[f] @ w.T  [NF, D]
    GWre = (c[:, None] * Gre) @ w64.T
    GWim = (c[:, None] * Gim) @ w64.T

    # lambda^2 per freq (only matters for fp8-GEMM pairs)
    lam2 = np.ones(NF)
    for pi in GEMM_F8:
        fs = pair_freqs[pi]
        gmax = np.maximum(
            np.abs(GWre[fs]).max(axis=1), np.abs(GWim[fs]).max(axis=1)
        )
        lam2[fs] = np.maximum(1.0, gmax * OUT_SC / (B8S * GW_TGT))

    # ---- CS tensors ----
    def block_cols(pi):
        fs = pair_freqs[pi]
        cos_m, sin_m = _cs_cols(PAIR_BLOCK[pi], fs)
        lam = np.sqrt(lam2[fs])
        return cos_m * lam[None, :], sin_m * lam[None, :]

    # EEE: chunk0 = [Re(0), Re(fs)], chunk1 = [Re(1024), Im(fs)]
    cos_e, sin_e = block_cols(0)
    cos_dc, _ = _cs_cols("EEE", [0])
    cos_ny, _ = _cs_cols("EEE", [NF - 1])
    cs_eee = np.concatenate([cos_dc, cos_e, cos_ny, sin_e], axis=1)  # [256, 256]
    cos_o, sin_o = block_cols(1)
    cs_eeo = np.concatenate([cos_o, sin_o], axis=1)
    # EO: chunks 4,5 = Re(top128, bot128); 6,7 = Im. fp16 tensor has chunks 4,6;
    # fp8 tensor has chunks 5,7.
    cos2, sin2 = block_cols(2)
    cos3, sin3 = block_cols(3)
    cs_eo16 = np.concatenate([cos2, sin2], axis=1)  # chunks 4, 6
    cs_eo8 = np.concatenate([cos3, sin3], axis=1)  # chunks 5, 7
    # O: chunks 8..11 = Re(rank blocks), 12..15 = Im. fp16: chunks 8,12,9,13
    # (pairs 4,5); fp8: 10,14,11,15 (pairs 6,7).
    cos4, sin4 = block_cols(4)
    cos5, sin5 = block_cols(5)
    cos6, sin6 = block_cols(6)
    cos7, sin7 = block_cols(7)
    cs_o16 = np.concatenate([cos4, sin4, cos5, sin5], axis=1)  # 8,12,9,13
    cs_o8 = np.concatenate([cos6, sin6, cos7, sin7], axis=1)  # 10,14,11,15

    consts = {
        "CSEEE": _chunk_cs(cs_eee, 2, 2).astype(np.float32).astype(f16),
        "CSEEO": _chunk_cs(cs_eeo, 2, 2).astype(np.float32).astype(f16),
        "CSEO16": _chunk_cs(cs_eo16, 2, 4).astype(np.float32).astype(f16),
        "CSO16": _chunk_cs(cs_o16, 4, 8).astype(np.float32).astype(f16),
        "CSEO8": _chunk_cs8(cs_eo8, 2, 4).astype(np.float32).astype(e4),
        "CSO8": _chunk_cs8(cs_o8, 4, 8).astype(np.float32).astype(e4),
    }

    # ---- GW tensors ----
    # row at (chunk, pos): fp16 chunks CH16 order; fp8 pairs GEMM_F8 order.
    gw16 = np.zeros((P, N16, D), np.float64)
    gw8 = np.zeros((P, N8, 2, D), np.float64)
    for pi in range(8):
        fs = pair_freqs[pi]
        pfa, pfb = PAIR_CHUNKS[pi]
        off = 1 if pi == 0 else 0
        if pi in GEMM_F8:
            j8 = P8_IDX[pi]
            sc = OUT_SC / (B8S * lam2[fs])
            gw8[off : off + len(fs), j8, 0, :] = GWre[fs] * sc[:, None]
            gw8[off : off + len(fs), j8, 1, :] = GWim[fs] * sc[:, None]
        else:
            sc = OUT_SC / (B16S * lam2[fs])
            gw16[off : off + len(fs), CH16_IDX[pfa], :] = GWre[fs] * sc[:, None]
            gw16[off : off + len(fs), CH16_IDX[pfb], :] = GWim[fs] * sc[:, None]
    # DC / Nyquist pinned rows (pair 0, pos 0): pure real
    gw16[0, CH16_IDX[0], :] = GWre[0] * (OUT_SC / B16S)
    gw16[0, CH16_IDX[1], :] = GWre[NF - 1] * (OUT_SC / B16S)

    assert np.abs(gw8).max() <= 240.0, f"fp8 GW overflow: {np.abs(gw8).max()}"
    consts["GW16"] = gw16.astype(np.float32).astype(f16)
    consts["GW8"] = gw8.astype(np.float32).astype(e4)
    consts["pair_freqs"] = pair_freqs
    consts["lam"] = np.sqrt(lam2)
    _CACHE["ckey"] = key
    _CACHE["consts"] = consts
    return consts


def _make_c0(F, consts):
    """Packed, lambda-scaled scan init from complex carry spectrum F [NF]."""
    c0 = np.zeros((P, NPF), np.float32)
    lam = consts["lam"]
    for pi in range(8):
        fs = consts["pair_freqs"][pi]
        pfa, pfb = PAIR_CHUNKS[pi]
        off = 1 if pi == 0 else 0
        c0[off : off + len(fs), pfa] = (F[fs].real * lam[fs]).astype(np.float32)
        c0[off : off + len(fs), pfb] = (F[fs].imag * lam[fs]).astype(np.float32)
    c0[0, 0] = F[0].real
    c0[0, 1] = F[NF - 1].real
    return c0


def kernel(x, queries, keyvalues, w_out):
    x = np.asarray(x, dtype=np.float32)
    queries = np.asarray(queries, dtype=np.float32)
    keyvalues = np.asarray(keyvalues, dtype=np.float32)
    w_out = np.asarray(w_out, dtype=np.float32)

    if "nc" not in _CACHE:
        _CACHE["nc"] = _build_nc()
    nc = _CACHE["nc"]
    consts = _constants(queries, keyvalues, w_out)

    in_maps = []
    shards = []
    for b in range(NB):
        for h in range(2):
            shards.append((b, h))
            xs = x[b, h * T : (h + 1) * T]  # [T, D]
            xT3 = np.ascontiguousarray(xs.T).reshape(ND, P, T).transpose(1, 0, 2)
            xTc = np.ascontiguousarray(
                xT3.reshape(P, ND, NSLAB, TSB).transpose(2, 0, 1, 3)
            ).astype(f16)
            if h == 0:
                c0 = np.zeros((P, NPF), np.float32)
            else:
                F = np.fft.rfft(x[b, :T].sum(axis=0).astype(np.float64))
                c0 = _make_c0(F, consts)
            in_maps.append(
                {
                    "xT": xTc,
                    "CSEEE": consts["CSEEE"],
                    "CSEEO": consts["CSEEO"],
                    "CSEO16": consts["CSEO16"],
                    "CSO16": consts["CSO16"],
                    "CSEO8": consts["CSEO8"],
                    "CSO8": consts["CSO8"],
                    "GW16": consts["GW16"],
                    "GW8": consts["GW8"],
                    "C0": np.ascontiguousarray(c0),
                }
            )

    global _LAST_IN_MAPS
    _LAST_IN_MAPS = in_maps
    res = run_bass_kernel_spmd(nc, in_maps, core_ids=list(range(8)))
    y = np.empty((NB, NS, D), np.float32)
    inv = 1.0 / OUT_SC
    for i, (b, h) in enumerate(shards):
        y[b, h * T : (h + 1) * T] = res.results[i]["out"].astype(np.float32) * inv
    return y


# revision 3
# speedup vs baseline: 1.0744x; 1.0623x over previous
"""HRR binding self-attention kernel for 8 trn2 NeuronCores.

Math: out = irfft(c * rfft(x) * cumsum_s(rfft(x))) @ w_out.T, c = queries*keyvalues.
rfft is linear, so the causal cumsum commutes into the frequency domain; irfft
is linear, so it fuses into the output Linear: out = qv^T @ GW with
GW = (c * Gf) @ w_out.T precomputed on host.

The forward DFT uses a three-level decimation-in-frequency split with every
twiddle folded into host-precomputed matrices:
  EEE = rfft_256(uuu)          covers freqs 8m        (uuu = uu1+uu2)
  EEO = DFT'_256(uud)          covers freqs 8m+4      (uud = uu1-uu2)
  EO  = DFT'_512(ud)           covers freqs 4m+2      (ud = u1-u2, u = x1+x2)
  O   = DFT'_1024(x1-x2)       covers odd freqs

Hybrid precision: everything fp16 (tensor engine runs fp16 at bf16 rate, and
fp16's m10 cuts the quantization error ~8x vs bf16), EXCEPT low-spectral-weight
basis rows which run as fp8e4 DoubleRow matmuls at 2x rate. The basis rows are
re-sorted within each DFT block by |c_f|^2 so the heavy frequencies concentrate
in designated fp16 chunk pairs; c^2 is so heavy-tailed that the fp8 pairs carry
only ~13% of the output norm (total ~1.6e-2 rel err vs the 2e-2 budget).
DoubleRow packs two 128-row contraction chunks per instruction:
out += sum_i w[:,i].T @ m[:,i] with w [K,2,M], m [K,2,N] slot-major.

Per-frequency scales lambda^2 (folded host-side into CS columns, C0 carry and
GW rows) keep both fp8 operands inside e4m3's +-240 range; the Q copy applies
a power-of-2 beta per chunk (2^-6 fp16 / 2^-15 fp8) so qv fits fp16/fp8e4,
inverted via GW. Output staged fp16 scaled by 2^-2; host multiplies back.

Sharding: 8 shards = (batch b in 0..3) x (seq half h in 0..1), 2048 tokens
each. h=1 shards get the first half's contribution as an initial carry,
computed on host as rfft(x[b, :2048].sum(0)).

Chunk pairs (Re-chunk, Im-chunk) at equal partition positions:
  pair 0 (0,1)=EEE f=8m [c-sorted, DC/Nyq pinned at pos 0], 1 (2,3)=EEO 8m+4,
  2 (4,6)/3 (5,7)=EO 4m+2 split by c-rank, 4..7 (8,12)(9,13)(10,14)(11,15)=O
  odd freqs by c-rank. GEMM fp8 pairs: {1,3,5,6,7}; DFT fp8 pairs: {3,5,6,7}.

Per-core single pass over 4 slabs of 512 tokens; emission interleaves slab
s's DFT with slab s-1's output matmul so the PE never idles."""

import sys

sys.path.insert(0, "/opt/trn_rl_repo")

import hashlib

import numpy as np
import ml_dtypes

import concourse.bass as bass
import concourse.bacc as bacc
import concourse.mybir as mybir
from concourse.tile import TileContext
from concourse.bass_utils import run_bass_kernel_spmd

F16 = mybir.dt.float16
FP8 = mybir.dt.float8e4
F32 = mybir.dt.float32
ADD = mybir.AluOpType.add
BYP = mybir.AluOpType.bypass
DR = mybir.MatmulPerfMode.DoubleRow

P = 128
D = 2048
T = 2048
ND = D // P
NPF = 16
TSB = 512
NSLAB = T // TSB
NB = 4
NS = 4096
NF = D // 2 + 1

f16 = np.float16
e4 = ml_dtypes.float8_e4m3

# ---- hybrid precision config ----
PAIR_CHUNKS = [(0, 1), (2, 3), (4, 6), (5, 7), (8, 12), (9, 13), (10, 14), (11, 15)]
PAIR_BLOCK = ["EEE", "EEO", "EO", "EO", "O", "O", "O", "O"]
BLOCK_US = {"EEE": (0, 2), "EEO": (2, 2), "EO": (4, 4), "O": (8, 8)}
GEMM_F8 = (1, 3, 5, 6, 7)  # pair indices with fp8 output GEMM
DFT_F8 = (3, 5, 6, 7)  # pair indices with fp8 DFT (subset of GEMM_F8)
PAIRS_16 = (0, 2, 4)  # fp16 GEMM pairs
CH16 = [0, 1, 4, 6, 8, 12]  # chunks of PAIRS_16, GW16/qv16 order
CH16_IDX = {ch: i for i, ch in enumerate(CH16)}
P8_IDX = {pi: j for j, pi in enumerate(GEMM_F8)}
N16 = len(CH16)
N8 = len(GEMM_F8)
B16S = 2.0 ** -6  # Q-copy scale, fp16 pairs
B8S = 2.0 ** -15  # Q-copy scale, fp8 pairs
OUT_SC = 2.0 ** -2  # folded into GW; host multiplies output back
GW_TGT = 200.0  # target max of fp8 GW rows (<=240)

# CS tensor chunk orders (tile index -> chunk id)
CS_O16_CH = [8, 12]
CS_EO8_CH = [5, 7]
CS_O8_CH = [10, 14, 11, 15, 9, 13]

_CACHE = {}


def _build_nc(reps: int = 1):
    nc = bacc.Bacc("TRN2", target_bir_lowering=False, debug=False, num_devices=8)
    xT = nc.dram_tensor("xT", [NSLAB, P, ND, TSB], F16, kind="ExternalInput")
    CSEEE = nc.dram_tensor("CSEEE", [2, P, 2, P], F16, kind="ExternalInput")
    CSEEO = nc.dram_tensor("CSEEO", [2, P, 2, P], F16, kind="ExternalInput")
    CSEO16 = nc.dram_tensor("CSEO16", [2, P, 4, P], F16, kind="ExternalInput")
    CSO16 = nc.dram_tensor("CSO16", [2, P, 8, P], F16, kind="ExternalInput")
    CSEO8 = nc.dram_tensor("CSEO8", [2, P, 2, 2, P], FP8, kind="ExternalInput")
    CSO8 = nc.dram_tensor("CSO8", [6, P, 4, 2, P], FP8, kind="ExternalInput")
    GW16 = nc.dram_tensor("GW16", [P, N16, D], F16, kind="ExternalInput")
    GW8 = nc.dram_tensor("GW8", [P, N8, 2, D], FP8, kind="ExternalInput")
    C0 = nc.dram_tensor("C0", [P, NPF], F32, kind="ExternalInput")
    out = nc.dram_tensor("out", [T, D], F16, kind="ExternalOutput")

    with TileContext(nc) as tc:
        with (
            tc.tile_pool(name="misc", bufs=1) as misc,
            tc.tile_pool(name="wts", bufs=1) as wpool,
        ):
            c0_sb = misc.tile([P, NPF], F32)
            nc.sync.dma_start(c0_sb[:], C0[:])

            # Loop-invariant weight loads, hoisted out of the reps loop.
            # CS first (needed by the first DFT), GW per e-block last (each
            # 512-col GEMM group only waits on its own quarter).
            cseee_sb = wpool.tile([P, 2, 2, P], F16)
            for pf in range(2):
                nc.sync.dma_start(cseee_sb[:, pf], CSEEE[pf])
            cseeo_sb = wpool.tile([P, 2, 2, P], F16)
            for pf in range(2):
                nc.sync.dma_start(cseeo_sb[:, pf], CSEEO[pf])
            cseo16_sb = wpool.tile([P, 2, 4, P], F16)
            for pf in range(2):
                nc.sync.dma_start(cseo16_sb[:, pf], CSEO16[pf])
            cso16_sb = wpool.tile([P, 2, 8, P], F16)
            for pf in range(2):
                nc.sync.dma_start(cso16_sb[:, pf], CSO16[pf])
            cseo8_sb = wpool.tile([P, 2, 2, 2, P], FP8)
            for pf in range(2):
                nc.sync.dma_start(cseo8_sb[:, pf], CSEO8[pf])
            cso8_sb = wpool.tile([P, 6, 4, 2, P], FP8)
            for pf in range(6):
                nc.sync.dma_start(cso8_sb[:, pf], CSO8[pf])
            gw16_e = []
            gw8_e = []
            for e in range(4):
                g16 = wpool.tile([P, N16, 512], F16, tag=f"gw16_{e}")
                for i in range(N16):
                    nc.sync.dma_start(g16[:, i, :], GW16[:, i, e * 512 : (e + 1) * 512])
                gw16_e.append(g16)
                g8 = wpool.tile([P, N8, 2, 512], FP8, tag=f"gw8_{e}")
                for j in range(N8):
                    nc.sync.dma_start(g8[:, j], GW8[:, j, :, e * 512 : (e + 1) * 512])
                gw8_e.append(g8)

            # cs lookup: chunk id -> (tile, index)
            cs16 = {}
            cs16[0], cs16[1] = (cseee_sb, 0), (cseee_sb, 1)
            cs16[2], cs16[3] = (cseeo_sb, 0), (cseeo_sb, 1)
            cs16[4], cs16[6] = (cseo16_sb, 0), (cseo16_sb, 1)
            for i, ch in enumerate(CS_O16_CH):
                cs16[ch] = (cso16_sb, i)
            cs8 = {}
            for i, ch in enumerate(CS_EO8_CH):
                cs8[ch] = (cseo8_sb, i)
            for i, ch in enumerate(CS_O8_CH):
                cs8[ch] = (cso8_sb, i)

            import contextlib

            loop_ctx = (
                tc.For_i(0, reps, 1, staggered_reset=True)
                if reps > 1
                else contextlib.nullcontext()
            )
            with loop_ctx:
                _body(nc, tc, c0_sb, cs16, cs8, gw16_e, gw8_e, xT, out)
    nc.finalize()
    return nc


# O-side pairs first: they depend only on the first u/s prep op, so the DFT
# pipeline starts while the deeper EEE/EEO prep chain is still running.
PAIR_ORDER = [4, 6, 5, 7, 0, 3, 1, 2]


def _body(nc, tc, c0_sb, cs16, cs8, gw16_e, gw8_e, xT, out):
    with (
        tc.tile_pool(name="xt", bufs=2) as xpool,
        tc.tile_pool(name="ut", bufs=1) as utpool,
        tc.tile_pool(name="uut", bufs=1) as uutpool,
        tc.tile_pool(name="us", bufs=2) as uspool,
        tc.tile_pool(name="us8", bufs=2) as us8pool,
        tc.tile_pool(name="qsb", bufs=3) as qpool,
        tc.tile_pool(name="qv", bufs=2) as qvpool,
        tc.tile_pool(name="qv8", bufs=2) as qv8pool,
        tc.tile_pool(name="carry", bufs=2) as cpool,
        tc.tile_pool(name="tmp", bufs=1) as tpool,
        tc.tile_pool(name="osb", bufs=3) as opool,
        tc.tile_pool(name="psD", bufs=6, space="PSUM") as psD,
        tc.tile_pool(name="psC", bufs=2, space="PSUM") as psC,
    ):
        carry_prev = None
        qv_prev = None
        qv8_prev = None
        for s in range(NSLAB + 1):
            if s < NSLAB:
                xt = xpool.tile([P, ND, TSB], F16, tag="xt")
                for qd in range(4):
                    nc.sync.dma_start(
                        xt[:, 4 * qd : 4 * qd + 4, :], xT[s, :, 4 * qd : 4 * qd + 4, :]
                    )
                # us chunks: 0..1 = uuu, 2..3 = uud, 4..7 = ud, 8..15 = x1-x2
                us = uspool.tile([P, NPF, TSB], F16, tag="us")
                nc.vector.tensor_sub(us[:, 8:16, :], xt[:, 0:8, :], xt[:, 8:16, :])
                u_t = utpool.tile([P, 8, TSB], F16, tag="ut")
                nc.vector.tensor_add(u_t[:], xt[:, 0:8, :], xt[:, 8:16, :])
                uu_t = uutpool.tile([P, 4, TSB], F16, tag="uut")
                nc.vector.tensor_add(uu_t[:], u_t[:, 0:4, :], u_t[:, 4:8, :])
                nc.vector.tensor_sub(us[:, 4:8, :], u_t[:, 0:4, :], u_t[:, 4:8, :])
                nc.vector.tensor_add(us[:, 0:2, :], uu_t[:, 0:2, :], uu_t[:, 2:4, :])
                nc.vector.tensor_sub(us[:, 2:4, :], uu_t[:, 0:2, :], uu_t[:, 2:4, :])
                # fp8 copies of us chunks 4..15 (EO + O contractions)
                us8 = us8pool.tile([P, 12, TSB], FP8, tag="us8")
                nc.vector.tensor_copy(us8[:, 4:12, :], us[:, 8:16, :])
                nc.vector.tensor_copy(us8[:, 0:4, :], us[:, 4:8, :])

                qv = qvpool.tile([P, N16, TSB], F16, tag="qv")
                qv8 = qv8pool.tile([P, N8, 2, TSB], FP8, tag="qv8")
                carry_sb = cpool.tile([P, NPF], F32, tag="carry")
                Qp0 = None
                for pi in PAIR_ORDER:
                    pfa, pfb = PAIR_CHUNKS[pi]
                    blk = PAIR_BLOCK[pi]
                    ob, ndc = BLOCK_US[blk]
                    isf8_dft = pi in DFT_F8
                    isf8_gemm = pi in GEMM_F8
                    beta = B8S if isf8_gemm else B16S
                    Qp = qpool.tile([P, 2, TSB], F16, tag="Q")
                    if pi == 0:
                        Qp0 = Qp
                    psts = {}
                    for h, pf in enumerate((pfa, pfb)):
                        pst = psD.tile([P, TSB], F32, tag="psD")
                        if isf8_dft:
                            tile8, idx8 = cs8[pf]
                            u8b = ob - 4  # us8 index base
                            for jdc in range(ndc // 2):
                                nc.tensor.matmul(
                                    pst[:],
                                    tile8[:, idx8, jdc],
                                    us8[:, u8b + 2 * jdc : u8b + 2 * jdc + 2, :],
                                    start=(jdc == 0),
                                    stop=(jdc == ndc // 2 - 1),
                                    perf_mode=DR,
                                )
                        else:
                            tile16, idx16 = cs16[pf]
                            for dc in range(ndc):
                                nc.tensor.matmul(
                                    pst[:],
                                    tile16[:, idx16, dc, :],
                                    us[:, ob + dc, :],
                                    start=(dc == 0),
                                    stop=(dc == ndc - 1),
                                )
                        nc.scalar.mul(Qp[:, h, :], pst[:], beta)
                        init = (
                            c0_sb[:, pf : pf + 1]
                            if s == 0
                            else carry_prev[:, pf : pf + 1]
                        )
                        # op1=bypass: state = data0 + state; data1 ignored
                        nc.vector.tensor_tensor_scan(
                            pst[:], pst[:], Qp[:, h, :], init, ADD, BYP
                        )
                        nc.scalar.copy(carry_sb[:, pf : pf + 1], pst[:, TSB - 1 : TSB])
                        psts[h] = pst
                    SR, SI = psts[0], psts[1]
                    QR, QI = Qp[:, 0, :], Qp[:, 1, :]
                    if isf8_gemm:
                        j8 = P8_IDX[pi]
                        dst_re = qv8[:, j8, 0, :]
                        dst_im = qv8[:, j8, 1, :]
                    else:
                        dst_re = qv[:, CH16_IDX[pfa], :]
                        dst_im = qv[:, CH16_IDX[pfb], :]
                    t1 = tpool.tile([P, TSB], F32, tag="t1")
                    t2 = tpool.tile([P, TSB], F32, tag="t2")
                    nc.vector.tensor_mul(t1[:], QR, SR[:])
                    nc.vector.tensor_mul(t2[:], QI, SI[:])
                    nc.vector.tensor_sub(dst_re, t1[:], t2[:])
                    t3 = tpool.tile([P, TSB], F32, tag="t1")
                    t4 = tpool.tile([P, TSB], F32, tag="t2")
                    nc.vector.tensor_mul(t3[:], QR, SI[:])
                    nc.vector.tensor_mul(t4[:], QI, SR[:])
                    nc.vector.tensor_add(dst_im, t3[:], t4[:])
                    if pi == 0:
                        # DC (chunk 0 pos 0) and Nyquist (chunk 1 pos 0): purely real
                        nc.vector.tensor_mul(qv[0:1, 0, :], Qp0[0:1, 0, :], SR[0:1, :])
                        nc.vector.tensor_mul(qv[0:1, 1, :], Qp0[0:1, 1, :], SI[0:1, :])
                carry_prev = carry_sb

            if s > 0:
                for tb in range(TSB // P):
                    for e in range(4):
                        psc = psC.tile([P, 512], F32, tag="psC")
                        # interleave fp16 / DoubleRow insts so the 256-row
                        # DR stationary loads hide behind fp16 streams
                        plan = []
                        for k in range(max(N16, N8)):
                            if k < N16:
                                plan.append((False, k))
                            if k < N8:
                                plan.append((True, k))
                        for n, (is8, k) in enumerate(plan):
                            if is8:
                                nc.tensor.matmul(
                                    psc[:],
                                    qv8_prev[:, k, :, tb * P : (tb + 1) * P],
                                    gw8_e[e][:, k],
                                    start=(n == 0),
                                    stop=(n == len(plan) - 1),
                                    perf_mode=DR,
                                )
                            else:
                                nc.tensor.matmul(
                                    psc[:],
                                    qv_prev[:, k, tb * P : (tb + 1) * P],
                                    gw16_e[e][:, k, :],
                                    start=(n == 0),
                                    stop=(n == len(plan) - 1),
                                )
                        osb = opool.tile([P, 512], F16, tag="osb")
                        if e % 2 == 0:
                            nc.scalar.copy(osb[:], psc[:])
                        else:
                            nc.vector.tensor_copy(osb[:], psc[:])
                        r0 = (s - 1) * TSB + tb * P
                        nc.sync.dma_start(
                            out[r0 : r0 + P, e * 512 : (e + 1) * 512], osb[:]
                        )
            if s < NSLAB:
                qv_prev = qv
                qv8_prev = qv8


def _cs_cols(block, freqs):
    """CS columns for given output freqs of a block: cos cols then -sin cols.
    Maps the block's prepped input (uuu/uud/ud/x1-x2) to Re/Im of rfft."""
    f = np.asarray(freqs, dtype=np.float64)
    if block == "EEE":
        d = np.arange(256, dtype=np.float64)
        ang = 2.0 * np.pi / 256 * np.outer(d, f / 8)
    elif block == "EEO":
        d = np.arange(256, dtype=np.float64)
        ang = np.pi / 256 * np.outer(d, 2 * (f - 4) / 8 + 1)
    elif block == "EO":
        d = np.arange(512, dtype=np.float64)
        ang = np.pi / 512 * np.outer(d, 2 * (f - 2) / 4 + 1)
    else:
        d = np.arange(1024, dtype=np.float64)
        ang = np.pi / 1024 * np.outer(d, 2 * (f - 1) / 2 + 1)
    return np.cos(ang), -np.sin(ang)


def _chunk_cs(mat, npf, ndc):
    """[d_in, n_cols] -> [npf, P, ndc, P]: entry [pf,p,dc,j] = mat[128*dc+p, 128*pf+j]."""
    return np.ascontiguousarray(
        mat.reshape(ndc, P, npf, P).transpose(2, 1, 0, 3)
    )


def _chunk_cs8(mat, npf, ndc):
    """[d_in, n_cols] -> [npf, P, ndc//2, 2, P] with dc pairs in the slot dim."""
    a = mat.reshape(ndc // 2, 2, P, npf, P)  # [jdc, slot, p, pf, j]
    return np.ascontiguousarray(a.transpose(3, 2, 0, 1, 4))


def _constants(queries, keyvalues, w_out):
    key = hashlib.sha1(
        queries.tobytes() + keyvalues.tobytes() + w_out.tobytes()
    ).hexdigest()
    if _CACHE.get("ckey") == key:
        return _CACHE["consts"]

    c = (queries.reshape(-1).astype(np.float64)) * (
        keyvalues.reshape(-1).astype(np.float64)
    )
    alpha = np.full(NF, 2.0)
    alpha[0] = alpha[NF - 1] = 1.0
    wgt = (c * alpha) ** 2

    def srt(fs):
        fs = np.asarray(fs)
        return fs[np.argsort(-wgt[fs], kind="stable")]

    f_eee = srt(8 * np.arange(1, 128))
    f_eeo = srt(8 * np.arange(128) + 4)
    f_eo = srt(4 * np.arange(256) + 2)
    f_o = srt(2 * np.arange(512) + 1)
    pair_freqs = {
        0: f_eee,
        1: f_eeo,
        2: f_eo[:128],
        3: f_eo[128:],
        4: f_o[:128],
        5: f_o[128:256],
        6: f_o[256:384],
        7: f_o[384:],
    }

    # G synthesis rows (irfft columns) [NF, D] for Re and Im parts
    dfull = np.arange(D, dtype=np.float64)
    ff = np.arange(NF, dtype=np.float64)
    ang = 2.0 * np.pi / D * np.outer(ff, dfull)  # [NF, D]
    Gre = (alpha[:, None] * np.cos(ang)) / D
    Gim = (-2.0 * np.sin(ang)) / D

    w64 = w_out.astype(np.float64)
    # GW natural rows per freq: re_row[f] = c_f * Gre[f] @ w.T  [NF, D]
    GWre = (c[:, None] * Gre) @ w64.T
    GWim = (c[:, None] * Gim) @ w64.T

    # lambda^2 per freq (only matters for fp8-GEMM pairs)
    lam2 = np.ones(NF)
    for pi in GEMM_F8:
        fs = pair_freqs[pi]
        gmax = np.maximum(
            np.abs(GWre[fs]).max(axis=1), np.abs(GWim[fs]).max(axis=1)
        )
        lam2[fs] = np.maximum(1.0, gmax * OUT_SC / (B8S * GW_TGT))

    # ---- CS tensors ----
    def block_cols(pi):
        fs = pair_freqs[pi]
        cos_m, sin_m = _cs_cols(PAIR_BLOCK[pi], fs)
        lam = np.sqrt(lam2[fs])
        return cos_m * lam[None, :], sin_m * lam[None, :]

    # EEE: chunk0 = [Re(0), Re(fs)], chunk1 = [Re(1024), Im(fs)]
    cos_e, sin_e = block_cols(0)
    cos_dc, _ = _cs_cols("EEE", [0])
    cos_ny, _ = _cs_cols("EEE", [NF - 1])
    cs_eee = np.concatenate([cos_dc, cos_e, cos_ny, sin_e], axis=1)  # [256, 256]
    cos_o, sin_o = block_cols(1)
    cs_eeo = np.concatenate([cos_o, sin_o], axis=1)
    # EO: chunks 4,5 = Re(top128, bot128); 6,7 = Im. fp16 tensor has chunks 4,6;
    # fp8 tensor has chunks 5,7.
    cos2, sin2 = block_cols(2)
    cos3, sin3 = block_cols(3)
    cs_eo16 = np.concatenate([cos2, sin2], axis=1)  # chunks 4, 6
    cs_eo8 = np.concatenate([cos3, sin3], axis=1)  # chunks 5, 7
    # O: chunks 8..11 = Re(rank blocks), 12..15 = Im. fp16: chunks 8,12,9,13
    # (pairs 4,5); fp8: 10,14,11,15 (pairs 6,7).
    cos4, sin4 = block_cols(4)
    cos5, sin5 = block_cols(5)
    cos6, sin6 = block_cols(6)
    cos7, sin7 = block_cols(7)
    cs_o16 = np.concatenate([cos4, sin4], axis=1)  # 8,12
    cs_o8 = np.concatenate([cos6, sin6, cos7, sin7, cos5, sin5], axis=1)  # 10,14,11,15,9,13

    consts = {
        "CSEEE": _chunk_cs(cs_eee, 2, 2).astype(np.float32).astype(f16),
        "CSEEO": _chunk_cs(cs_eeo, 2, 2).astype(np.float32).astype(f16),
        "CSEO16": _chunk_cs(cs_eo16, 2, 4).astype(np.float32).astype(f16),
        "CSO16": _chunk_cs(cs_o16, 2, 8).astype(np.float32).astype(f16),
        "CSEO8": _chunk_cs8(cs_eo8, 2, 4).astype(np.float32).astype(e4),
        "CSO8": _chunk_cs8(cs_o8, 6, 8).astype(np.float32).astype(e4),
    }

    # ---- GW tensors ----
    # row at (chunk, pos): fp16 chunks CH16 order; fp8 pairs GEMM_F8 order.
    gw16 = np.zeros((P, N16, D), np.float64)
    gw8 = np.zeros((P, N8, 2, D), np.float64)
    for pi in range(8):
        fs = pair_freqs[pi]
        pfa, pfb = PAIR_CHUNKS[pi]
        off = 1 if pi == 0 else 0
        if pi in GEMM_F8:
            j8 = P8_IDX[pi]
            sc = OUT_SC / (B8S * lam2[fs])
            gw8[off : off + len(fs), j8, 0, :] = GWre[fs] * sc[:, None]
            gw8[off : off + len(fs), j8, 1, :] = GWim[fs] * sc[:, None]
        else:
            sc = OUT_SC / (B16S * lam2[fs])
            gw16[off : off + len(fs), CH16_IDX[pfa], :] = GWre[fs] * sc[:, None]
            gw16[off : off + len(fs), CH16_IDX[pfb], :] = GWim[fs] * sc[:, None]
    # DC / Nyquist pinned rows (pair 0, pos 0): pure real
    gw16[0, CH16_IDX[0], :] = GWre[0] * (OUT_SC / B16S)
    gw16[0, CH16_IDX[1], :] = GWre[NF - 1] * (OUT_SC / B16S)

    assert np.abs(gw8).max() <= 240.0, f"fp8 GW overflow: {np.abs(gw8).max()}"
    consts["GW16"] = gw16.astype(np.float32).astype(f16)
    consts["GW8"] = gw8.astype(np.float32).astype(e4)
    consts["pair_freqs"] = pair_freqs
    consts["lam"] = np.sqrt(lam2)
    _CACHE["ckey"] = key
    _CACHE["consts"] = consts
    return consts


def _make_c0(F, consts):
    """Packed, lambda-scaled scan init from complex carry spectrum F [NF]."""
    c0 = np.zeros((P, NPF), np.float32)
    lam = consts["lam"]
    for pi in range(8):
        fs = consts["pair_freqs"][pi]
        pfa, pfb = PAIR_CHUNKS[pi]
        off = 1 if pi == 0 else 0
        c0[off : off + len(fs), pfa] = (F[fs].real * lam[fs]).astype(np.float32)
        c0[off : off + len(fs), pfb] = (F[fs].imag * lam[fs]).astype(np.float32)
    c0[0, 0] = F[0].real
    c0[0, 1] = F[NF - 1].real
    return c0


def kernel(x, queries, keyvalues, w_out):
    x = np.asarray(x, dtype=np.float32)
    queries = np.asarray(queries, dtype=np.float32)
    keyvalues = np.asarray(keyvalues, dtype=np.float32)
    w_out = np.asarray(w_out, dtype=np.float32)

    if "nc" not in _CACHE:
        _CACHE["nc"] = _build_nc()
    nc = _CACHE["nc"]
    consts = _constants(queries, keyvalues, w_out)

    in_maps = []
    shards = []
    for b in range(NB):
        for h in range(2):
            shards.append((b, h))
            xs = x[b, h * T : (h + 1) * T]  # [T, D]
            xT3 = np.ascontiguousarray(xs.T).reshape(ND, P, T).transpose(1, 0, 2)
            xTc = np.ascontiguousarray(
                xT3.reshape(P, ND, NSLAB, TSB).transpose(2, 0, 1, 3)
            ).astype(f16)
            if h == 0:
                c0 = np.zeros((P, NPF), np.float32)
            else:
                F = np.fft.rfft(x[b, :T].sum(axis=0).astype(np.float64))
                c0 = _make_c0(F, consts)
            in_maps.append(
                {
                    "xT": xTc,
                    "CSEEE": consts["CSEEE"],
                    "CSEEO": consts["CSEEO"],
                    "CSEO16": consts["CSEO16"],
                    "CSO16": consts["CSO16"],
                    "CSEO8": consts["CSEO8"],
                    "CSO8": consts["CSO8"],
                    "GW16": consts["GW16"],
                    "GW8": consts["GW8"],
                    "C0": np.ascontiguousarray(c0),
                }
            )

    global _LAST_IN_MAPS
    _LAST_IN_MAPS = in_maps
    res = run_bass_kernel_spmd(nc, in_maps, core_ids=list(range(8)))
    y = np.empty((NB, NS, D), np.float32)
    inv = 1.0 / OUT_SC
    for i, (b, h) in enumerate(shards):
        y[b, h * T : (h + 1) * T] = res.results[i]["out"].astype(np.float32) * inv
    return y
